# revision 9
# baseline (speedup 1.0000x reference)
"""MeshPool segment-mean kernel for Trainium2 (8 NeuronCores, batch-parallel).

Per core (one mesh b): out[c, t] = mean over edges e with group_ids[e] == t of
fe[c, e]; fe is [256, 18000] fp32, 4500 target groups.

Device algorithm (all feature math on device):
  Phase 1: PE-transpose fe [256, E] -> feT [E, 256] in HBM scratch.
  Phase 2: edges are binned (CPU, index-only) by target-window (36 windows of
    128 targets, CWIN 128-slot chunks per window).  Per window: dma_gather the
    member edge rows of feT into SBUF slots, build per-chunk one-hot selector
    matrices from local target ids (iota==lid compare on DVE), and reduce with
    PE matmuls accumulating in PSUM: sums = onehot^T @ gathered, counts =
    onehot^T @ ones.  Divide by max(counts,1) (reciprocal+scale), PE-transpose
    back to [channel, target] and stream to the output.

CPU does index marshalling only: stable argsort of group_ids into window bins,
gather index tables, local ids, int16 packing.  All sums/counts/means are
computed on device in fp32 (PE fp32 matmul with exact 0/1 one-hots).
"""
import sys
sys.path.insert(0, "/opt/trn_rl_repo")
import numpy as np

B, C, E, T = 8, 256, 18000, 4500
NWIN = 36                 # target windows of 128 (last has 20 targets)
E_PAD = 18048
OUTW = NWIN * 128         # 4608 outbuf columns (>= T)
_cache = {}
STORAGE = "bf16"          # feT/gather/one-hot matmul storage dtype


def build(cwin=5, reps=1, storage="bf16"):
    import contextlib
    import concourse.bacc as bacc
    import concourse.mybir as mybir
    from concourse.tile import TileContext
    from concourse import masks

    f32 = mybir.dt.float32
    i16 = mybir.dt.int16
    dt_s = {"bf16": mybir.dt.bfloat16, "f32": f32,
            "f32r": mybir.dt.float32r}[storage]
    eq = mybir.AluOpType.is_equal
    NSLOT = NWIN * cwin * 128
    NCHUNK = NWIN * cwin

    nc = bacc.Bacc(None, target_bir_lowering=False, num_swdge_queues=2)
    fe = nc.dram_tensor("fe", [C, E], f32, kind="ExternalInput")
    gidx = nc.dram_tensor("gidx", [128, NSLOT // 16], i16, kind="ExternalInput")
    lids = nc.dram_tensor("lids", [128, NCHUNK], f32, kind="ExternalInput")
    out = nc.dram_tensor("out", [C, T], f32, kind="ExternalOutput")
    feT = nc.dram_tensor("feT", [E_PAD, C], dt_s, kind="Internal")

    with TileContext(nc) as tc:
        with tc.tile_pool(name="const", bufs=1) as constp:
            ident = constp.tile([128, 128], f32)
            masks.make_identity(nc, ident[:])
            iota = constp.tile([128, 128], f32)
            nc.gpsimd.iota(iota[:], [[1, 128]], channel_multiplier=0,
                           allow_small_or_imprecise_dtypes=True)
            ones = constp.tile([128, 1], dt_s)
            nc.vector.memset(ones[:], 1.0)
            tgidx = constp.tile([128, NSLOT // 16], i16)
            nc.sync.dma_start(tgidx[:], gidx[:])
            tlids = constp.tile([128, NCHUNK], f32)
            nc.sync.dma_start(tlids[:], lids[:])
            o0 = constp.tile([128, OUTW], f32)
            o1 = constp.tile([128, OUTW], f32)

            rep_ctx = tc.For_i(0, reps) if reps > 1 else contextlib.nullcontext()
            with rep_ctx:
                # ---------------- Phase 1: fe -> feT (PE transpose) ----------
                with (
                    tc.tile_pool(name="p1", bufs=3) as p1,
                    tc.tile_pool(name="pp1", bufs=2, space="PSUM") as pp1,
                ):
                    ECH = 512
                    nch1 = (E + ECH - 1) // ECH          # 36 (last chunk 80)
                    for k in range(nch1):
                        base = k * ECH
                        we = min(ECH, E - base)
                        nsub = (we + 127) // 128
                        a0 = p1.tile([128, ECH], f32, tag="a0")
                        a1 = p1.tile([128, ECH], f32, tag="a1")
                        nc.sync.dma_start(a0[:, 0:we], fe[0:128, base:base + we])
                        nc.sync.dma_start(a1[:, 0:we], fe[128:256, base:base + we])
                        ps = pp1.tile([128, 4, 256], f32, tag="ps")
                        st = p1.tile([128, 4, 256], dt_s, tag="st")
                        for j in range(nsub):
                            wj = min(128, we - j * 128)
                            sl = slice(j * 128, j * 128 + wj)
                            nc.tensor.transpose(ps[0:wj, j, 0:128], a0[:, sl], ident[:])
                            nc.tensor.transpose(ps[0:wj, j, 128:256], a1[:, sl], ident[:])
                        cp = nc.vector.tensor_copy if k % 2 == 0 else nc.scalar.copy
                        cp(st[:, 0:nsub, :], ps[:, 0:nsub, :])
                        if we == ECH:
                            dst = fe_rows = feT[base:base + we, :].rearrange(
                                "(j p) c -> p j c", p=128)
                            nc.sync.dma_start(dst, st[:, 0:nsub, :])
                        else:
                            for j in range(nsub):
                                wj = min(128, we - j * 128)
                                nc.sync.dma_start(
                                    feT[base + j * 128:base + j * 128 + wj, :],
                                    st[0:wj, j, :])

                # ---------------- Phase 2: gather + one-hot reduce -----------
                with (
                    tc.tile_pool(name="gp", bufs=3) as gp,
                    tc.tile_pool(name="up", bufs=4) as up,
                    tc.tile_pool(name="rp", bufs=2) as rp,
                    tc.tile_pool(name="ps_s", bufs=2, space="PSUM") as psp_s,
                    tc.tile_pool(name="ps_c", bufs=2, space="PSUM") as psp_c,
                    tc.tile_pool(name="ps_t", bufs=2, space="PSUM") as psp_t,
                ):
                    for w in range(NWIN):
                        g = gp.tile([128, cwin, 256], dt_s, tag="g")
                        nc.gpsimd.dma_gather(
                            g[:], feT[:],
                            tgidx[:, w * cwin * 8:(w + 1) * cwin * 8],
                            cwin * 128, cwin * 128, 256,
                            queue_num=w % 2,
                        )
                        ps_s = psp_s.tile([128, 256], f32, tag="s")
                        ps_c = psp_c.tile([128, 2], f32, tag="c")
                        for j in range(cwin):
                            ch = w * cwin + j
                            U = up.tile([128, 128], dt_s, tag="U")
                            nc.vector.tensor_scalar(
                                U[:], iota[:], tlids[:, ch:ch + 1], None, op0=eq)
                            nc.tensor.matmul(ps_s[:], U[:], g[:, j, :],
                                             start=(j == 0), stop=(j == cwin - 1))
                            nc.tensor.matmul(ps_c[:, 0:1], U[:], ones[:],
                                             start=(j == 0), stop=(j == cwin - 1))
                        cnt = rp.tile([128, 1], f32, tag="cnt")
                        nc.vector.tensor_scalar_max(cnt[:], ps_c[:, 0:1], 1.0)
                        rec = rp.tile([128, 1], f32, tag="rec")
                        nc.vector.reciprocal(rec[:], cnt[:])
                        res = rp.tile([128, 256], f32, tag="res")
                        nc.vector.tensor_scalar_mul(res[:], ps_s[:], rec[:])
                        ps_t = psp_t.tile([128, 256], f32, tag="t")
                        nc.tensor.transpose(ps_t[:, 0:128], res[:, 0:128], ident[:])
                        nc.tensor.transpose(ps_t[:, 128:256], res[:, 128:256], ident[:])
                        csl = slice(w * 128, (w + 1) * 128)
                        nc.vector.tensor_copy(o0[:, csl], ps_t[:, 0:128])
                        nc.scalar.copy(o1[:, csl], ps_t[:, 128:256])

                nc.sync.dma_start(out[0:128, :], o0[:, 0:T])
                nc.sync.dma_start(out[128:256, :], o1[:, 0:T])
    nc.finalize()
    return nc


def prep_indices(gids, cwin):
    """CPU index marshalling: bin edges by target-window, build gather table +
    local target ids. gids: [E] ints in [0, T)."""
    NSLOT = NWIN * cwin * 128
    order = np.argsort(gids, kind="stable")
    win = gids[order] >> 7
    idx_slots = np.zeros(NSLOT, np.int16)
    lid_slots = np.full(NSLOT, -7.0, np.float32)
    start = np.searchsorted(win, np.arange(NWIN))
    end = np.searchsorted(win, np.arange(NWIN), side="right")
    for w in range(NWIN):
        sel = order[start[w]:end[w]]
        n = sel.size
        if n > cwin * 128:
            return None  # capacity exceeded; caller rebuilds with larger cwin
        base = w * cwin * 128
        idx_slots[base:base + n] = sel.astype(np.int16)
        lid_slots[base:base + n] = (gids[sel] - (w << 7)).astype(np.float32)
    gidx = np.tile(np.ascontiguousarray(idx_slots.reshape(-1, 16).T), (8, 1))
    lids = np.ascontiguousarray(lid_slots.reshape(-1, 128).T)
    return gidx, lids


def kernel(fe, group_ids, target):
    from concourse.bass_utils import run_bass_kernel_spmd

    fe = np.asarray(fe, dtype=np.float32)
    gids = np.asarray(group_ids).astype(np.int64)
    assert int(np.asarray(target)) == T and fe.shape == (B, C, E)

    cwin = 5
    while True:
        packed = [prep_indices(gids[b], cwin) for b in range(B)]
        if all(p is not None for p in packed):
            break
        cwin += 1
    key = (cwin, STORAGE)
    if key not in _cache:
        _cache[key] = build(cwin=cwin, storage=STORAGE)
    nc = _cache[key]
    in_maps = [
        {"fe": fe[b], "gidx": packed[b][0], "lids": packed[b][1]}
        for b in range(B)
    ]
    res = run_bass_kernel_spmd(nc, in_maps, core_ids=list(range(B)))
    return np.stack([res.results[b]["out"] for b in range(B)]).astype(np.float32)


# revision 11
# speedup vs baseline: 5.4629x; 5.4629x over previous
"""MeshPool segment-mean kernel for Trainium2 (8 NeuronCores, batch-parallel).

Per core (one mesh b): out[c, t] = mean over edges e with group_ids[e] == t of
fe[c, e]; fe is [256, 18000] fp32, 4500 target groups.

Device algorithm (all feature math on device):
  Phase 1: PE-transpose fe [256, E] -> feT [E, 256] in HBM scratch.
  Phase 2: edges are binned (CPU, index-only) by target-window (36 windows of
    128 targets, CWIN 128-slot chunks per window).  Per window: dma_gather the
    member edge rows of feT into SBUF slots, build per-chunk one-hot selector
    matrices from local target ids (iota==lid compare on DVE), and reduce with
    PE matmuls accumulating in PSUM: sums = onehot^T @ gathered, counts =
    onehot^T @ ones.  Divide by max(counts,1) (reciprocal+scale), PE-transpose
    back to [channel, target] and stream to the output.

CPU does index marshalling only: stable argsort of group_ids into window bins,
gather index tables, local ids, int16 packing.  All sums/counts/means are
computed on device in fp32 (PE fp32 matmul with exact 0/1 one-hots).
"""
import sys
sys.path.insert(0, "/opt/trn_rl_repo")
import numpy as np

B, C, E, T = 8, 256, 18000, 4500
NWIN = 36                 # target windows of 128 (last has 20 targets)
E_PAD = 18048
OUTW = NWIN * 128         # 4608 outbuf columns (>= T)
_cache = {}
STORAGE = "bf16"          # feT/gather/one-hot matmul storage dtype


def build(cwin=5, reps=1, storage="bf16"):
    import contextlib
    import concourse.bacc as bacc
    import concourse.mybir as mybir
    from concourse.tile import TileContext
    from concourse import masks

    f32 = mybir.dt.float32
    i16 = mybir.dt.int16
    dt_s = {"bf16": mybir.dt.bfloat16, "f32": f32,
            "f32r": mybir.dt.float32r}[storage]
    eq = mybir.AluOpType.is_equal
    NSLOT = NWIN * cwin * 128
    NCHUNK = NWIN * cwin

    nc = bacc.Bacc(None, target_bir_lowering=False, num_swdge_queues=2)
    fe = nc.dram_tensor("fe", [C, E], f32, kind="ExternalInput")
    gidx = nc.dram_tensor("gidx", [128, NSLOT // 16], i16, kind="ExternalInput")
    lids = nc.dram_tensor("lids", [128, NCHUNK], f32, kind="ExternalInput")
    out = nc.dram_tensor("out", [C, T], f32, kind="ExternalOutput")
    feT = nc.dram_tensor("feT", [E_PAD, C], dt_s, kind="Internal")

    with TileContext(nc) as tc:
        with tc.tile_pool(name="const", bufs=1) as constp:
            ident = constp.tile([128, 128], f32)
            masks.make_identity(nc, ident[:])
            iota = constp.tile([128, 128], f32)
            nc.gpsimd.iota(iota[:], [[1, 128]], channel_multiplier=0,
                           allow_small_or_imprecise_dtypes=True)
            ones = constp.tile([128, 1], dt_s)
            nc.vector.memset(ones[:], 1.0)
            tgidx = constp.tile([128, NSLOT // 16], i16)
            nc.sync.dma_start(tgidx[:], gidx[:])
            tlids = constp.tile([128, NCHUNK], f32)
            nc.sync.dma_start(tlids[:], lids[:])
            o0 = constp.tile([128, OUTW], f32)
            o1 = constp.tile([128, OUTW], f32)

            rep_ctx = tc.For_i(0, reps) if reps > 1 else contextlib.nullcontext()
            with rep_ctx:
                # ---------------- Phase 1: fe -> feT (PE transpose) ----------
                with (
                    tc.tile_pool(name="p1", bufs=4) as p1,
                    tc.tile_pool(name="pp1", bufs=2, space="PSUM") as pp1,
                ):
                    ECH = 512
                    nch1 = (E + ECH - 1) // ECH          # 36 (last chunk 80)
                    for k in range(nch1):
                        base = k * ECH
                        we = min(ECH, E - base)
                        nsub = (we + 127) // 128
                        a0 = p1.tile([128, ECH], f32, tag="a0")
                        a1 = p1.tile([128, ECH], f32, tag="a1")
                        nc.sync.dma_start(a0[:, 0:we], fe[0:128, base:base + we])
                        nc.sync.dma_start(a1[:, 0:we], fe[128:256, base:base + we])
                        ps = pp1.tile([128, 4, 256], f32, tag="ps")
                        st = p1.tile([128, 4, 256], dt_s, tag="st")
                        for j in range(nsub):
                            wj = min(128, we - j * 128)
                            sl = slice(j * 128, j * 128 + wj)
                            nc.tensor.transpose(ps[0:wj, j, 0:128], a0[:, sl], ident[:])
                            nc.tensor.transpose(ps[0:wj, j, 128:256], a1[:, sl], ident[:])
                        cp = nc.vector.tensor_copy if k % 2 == 0 else nc.scalar.copy
                        cp(st[:, 0:nsub, :], ps[:, 0:nsub, :])
                        if we == ECH:
                            dst = fe_rows = feT[base:base + we, :].rearrange(
                                "(j p) c -> p j c", p=128)
                            nc.sync.dma_start(dst, st[:, 0:nsub, :])
                        else:
                            for j in range(nsub):
                                wj = min(128, we - j * 128)
                                nc.sync.dma_start(
                                    feT[base + j * 128:base + j * 128 + wj, :],
                                    st[0:wj, j, :])

                # ---------------- Phase 2: gather + one-hot reduce -----------
                with (
                    tc.tile_pool(name="gp", bufs=4) as gp,
                    tc.tile_pool(name="up", bufs=6) as up,
                    tc.tile_pool(name="rp", bufs=2) as rp,
                    tc.tile_pool(name="ps_s", bufs=2, space="PSUM") as psp_s,
                    tc.tile_pool(name="ps_c", bufs=2, space="PSUM") as psp_c,
                    tc.tile_pool(name="ps_t", bufs=3, space="PSUM") as psp_t,
                ):
                    for w in range(NWIN):
                        g = gp.tile([128, cwin, 256], dt_s, tag="g")
                        nc.gpsimd.dma_gather(
                            g[:], feT[:],
                            tgidx[:, w * cwin * 8:(w + 1) * cwin * 8],
                            cwin * 128, cwin * 128, 256,
                            queue_num=w % 2,
                        )
                        ps_s = psp_s.tile([128, 256], f32, tag="s")
                        ps_c = psp_c.tile([128, 2], f32, tag="c")
                        for j in range(cwin):
                            ch = w * cwin + j
                            U = up.tile([128, 128], dt_s, tag="U")
                            nc.vector.tensor_scalar(
                                U[:], iota[:], tlids[:, ch:ch + 1], None, op0=eq)
                            nc.tensor.matmul(ps_s[:], U[:], g[:, j, :],
                                             start=(j == 0), stop=(j == cwin - 1))
                            nc.tensor.matmul(ps_c[:, 0:1], U[:], ones[:],
                                             start=(j == 0), stop=(j == cwin - 1))
                        cnt = rp.tile([128, 1], f32, tag="cnt")
                        nc.vector.tensor_scalar_max(cnt[:], ps_c[:, 0:1], 1.0)
                        rec = rp.tile([128, 1], f32, tag="rec")
                        nc.vector.reciprocal(rec[:], cnt[:])
                        res = rp.tile([128, 256], f32, tag="res")
                        nc.vector.tensor_scalar_mul(res[:], ps_s[:], rec[:])
                        ps_t = psp_t.tile([128, 256], f32, tag="t")
                        nc.tensor.transpose(ps_t[:, 0:128], res[:, 0:128], ident[:])
                        nc.tensor.transpose(ps_t[:, 128:256], res[:, 128:256], ident[:])
                        csl = slice(w * 128, (w + 1) * 128)
                        nc.vector.tensor_copy(o0[:, csl], ps_t[:, 0:128])
                        nc.scalar.copy(o1[:, csl], ps_t[:, 128:256])

                nc.sync.dma_start(out[0:128, :], o0[:, 0:T])
                nc.sync.dma_start(out[128:256, :], o1[:, 0:T])
    nc.finalize()
    return nc


def prep_indices(gids, cwin):
    """CPU index marshalling: bin edges by target-window, build gather table +
    local target ids. gids: [E] ints in [0, T)."""
    NSLOT = NWIN * cwin * 128
    order = np.argsort(gids, kind="stable")
    win = gids[order] >> 7
    idx_slots = np.zeros(NSLOT, np.int16)
    lid_slots = np.full(NSLOT, -7.0, np.float32)
    start = np.searchsorted(win, np.arange(NWIN))
    end = np.searchsorted(win, np.arange(NWIN), side="right")
    for w in range(NWIN):
        sel = order[start[w]:end[w]]
        n = sel.size
        if n > cwin * 128:
            return None  # capacity exceeded; caller rebuilds with larger cwin
        base = w * cwin * 128
        idx_slots[base:base + n] = sel.astype(np.int16)
        lid_slots[base:base + n] = (gids[sel] - (w << 7)).astype(np.float32)
    gidx = np.tile(np.ascontiguousarray(idx_slots.reshape(-1, 16).T), (8, 1))
    lids = np.ascontiguousarray(lid_slots.reshape(-1, 128).T)
    return gidx, lids


def kernel(fe, group_ids, target):
    from concourse.bass_utils import run_bass_kernel_spmd

    fe = np.asarray(fe, dtype=np.float32)
    gids = np.asarray(group_ids).astype(np.int64)
    assert int(np.asarray(target)) == T and fe.shape == (B, C, E)

    cwin = 5
    while True:
        packed = [prep_indices(gids[b], cwin) for b in range(B)]
        if all(p is not None for p in packed):
            break
        cwin += 1
    key = (cwin, STORAGE)
    if key not in _cache:
        _cache[key] = build(cwin=cwin, storage=STORAGE)
    nc = _cache[key]
    in_maps = [
        {"fe": fe[b], "gidx": packed[b][0], "lids": packed[b][1]}
        for b in range(B)
    ]
    res = run_bass_kernel_spmd(nc, in_maps, core_ids=list(range(B)))
    return np.stack([res.results[b]["out"] for b in range(B)]).astype(np.float32)
